# revision 1
# baseline (speedup 1.0000x reference)
"""BFP-quantized 3x3 conv (nn_BFConv2d) on 8 TRN2 NeuronCores.

Strategy (data-parallel over batch, 4 samples/core):
  Program A (quantize): per core, for each of its 4 samples, load a
    group-aligned window of the flattened x (the BFP group grid is global
    over the flat tensor; each per-sample window starts on a 36-element
    group boundary, so the in-kernel grid is exact), compute the BFP
    quantization with the magic-number trick
        q = (x + M) - M,  M = 1.5 * 2^23 * scale = exp_bits(absmax) * 98304
    (exact round-half-even onto the group lattice; results are <=9
    significant bits so bf16 is exact), and write q as bf16. The weight
    tensor (36864 elems = exactly 1024 groups) is quantized the same way.
  Host: slice each sample's quantized window by its group-grid phase
    (pre in [0,36)) to get slab-aligned q; pure numpy, no device work.
  Program B (conv): 3x3 conv as 9 shifted 64x64 bf16 matmuls per output
    tile, using TensorE 64x64 array tiling: quadrant (0,0) processes the
    even sample of a pair (SBUF partitions 0-63), quadrant (64,64) the odd
    sample (partitions 64-127), both accumulating into one PSUM bank.
    ScalarE evacuates PSUM with the bias add fused; one full-width DMA
    writes both samples' rows (64*12544 == 802816 makes the pair layout
    contiguous in NCHW).
"""

import os
import sys
from contextlib import ExitStack

import numpy as np

sys.path.insert(0, "/opt/trn_rl_repo")

import ml_dtypes  # noqa: E402
import concourse.bacc as bacc  # noqa: E402
import concourse.mybir as mybir  # noqa: E402
import concourse.tile as tile  # noqa: E402

F32 = mybir.dt.float32
BF16 = mybir.dt.bfloat16
I32 = mybir.dt.int32

N_CORES = 8
B = 32                      # batch
C = 64                      # channels (in == out)
H = W = 112
SAMPLE = C * H * W          # 802816 elems per sample
GS = 36                     # BFP group size
GPP = 175                   # groups per partition in the quantize window
QCOLS = GPP * GS            # 6300
QWIN = 128 * QCOLS          # 806400 elems: covers a sample + phase slack
WP = W + 2                  # padded row width 114
XPAD = WP * WP + 2          # padded sample + 2 guard slots
MAGIC_MUL = 98304.0         # 1.5 * 2^16:  exp2(e) * this == 1.5*2^23*2^(e-7)

_cache = {}
last_exec_ns = {}
last_results = {}


def _ensure_snap_op():
    """Register a custom DVE op BFP_SNAP_ANT: out = (in0 + in1) - in1.

    One streaming pass for the BFP magic-number snap (vs add + subtract as
    two scalar_tensor_tensor passes). The per-NEFF DVE table machinery picks
    it up from dve_ops.OPS; sha is pinned from this environment's lowering.
    """
    import concourse.dve_ops as dops
    if getattr(dops, "_BFP_SNAP_ANT", None) is not None:
        return dops._BFP_SNAP_ANT
    from concourse.dve_spec import Spec, Src0, Src1, lower as spec_lower
    from concourse.dve_uop import DveOpSpec

    def _snap_ref(in0, in1, s0, s1, imm2):
        a = in0.astype(np.float32)
        b = np.broadcast_to(in1.astype(np.float32), in1.shape).reshape(a.shape)
        return (a + b) - b

    spec = Spec(body=(Src0 + Src1) - Src1, reference=_snap_ref)
    op = dops.DveOp("BFP_SNAP_ANT", spec, subdim=False, uops_sha={})
    idx = max(dops._SUB_OPCODE_FOR_NAME.values()) + 1
    assert idx < 0x20
    dops.OPS.append(op)
    dops.CUSTOM_DVE_SPECS["BFP_SNAP_ANT"] = spec
    dops._SUB_OPCODE_FOR_NAME["BFP_SNAP_ANT"] = idx
    for ver in ("v3", "v4"):
        try:
            s = DveOpSpec(name=op.name, opcode=idx,
                          uops=spec_lower(spec, ver=ver), rd1_en=True)
            op.uops_sha[ver] = s.sha(ver)
        except Exception:
            pass
    dops._BFP_SNAP_ANT = op
    return op


def _trace_enabled():
    return os.environ.get("BFP_TRACE") == "1"


def _install_trace_shim():
    """Provide antenv.axon_hooks (NTFF profiling hook) if the image lacks it.

    Mirrors trn_agent_boot.trn_boot._ntff_profile_via_ctypes: drives NRT
    profiling through the axon PJRT .so so run_bass_kernel_spmd(trace=True)
    can report HW exec time.
    """
    import types
    import ctypes
    import contextlib
    try:
        from antenv.axon_hooks import get_axon_ntff_profile_hook  # noqa: F401
        return
    except ImportError:
        pass
    so_path = "/opt/axon/libaxon_pjrt.so"
    if not os.path.exists(so_path):
        return
    lib = ctypes.CDLL(so_path)
    if not hasattr(lib, "axon_start_nrt_profile"):
        return
    lib.axon_start_nrt_profile.argtypes = [ctypes.POINTER(ctypes.c_int64),
                                           ctypes.c_size_t]
    lib.axon_start_nrt_profile.restype = ctypes.c_int64
    lib.axon_stop_nrt_profile.argtypes = [ctypes.c_char_p]
    lib.axon_stop_nrt_profile.restype = ctypes.c_int64

    @contextlib.contextmanager
    def _hook(output_dir, device_ids):
        import jax
        jax.devices()
        if device_ids:
            ids = (ctypes.c_int64 * len(device_ids))(*device_ids)
            rc = lib.axon_start_nrt_profile(ids, len(device_ids))
        else:
            rc = lib.axon_start_nrt_profile(None, 0)
        if rc != 0:
            raise RuntimeError(f"axon_start_nrt_profile rc={rc}")
        try:
            yield
        finally:
            n = lib.axon_stop_nrt_profile(str(output_dir).encode())
            print(f"profile: {n} ntff file(s) -> {output_dir}", file=sys.stderr)

    mod = types.ModuleType("antenv.axon_hooks")
    state = {"hook": _hook}
    mod.get_axon_ntff_profile_hook = lambda: state["hook"]
    mod.set_axon_ntff_profile_hook = lambda h: state.update(hook=h)
    sys.modules["antenv.axon_hooks"] = mod
    import antenv
    antenv.axon_hooks = mod
    from concourse import bass_utils as bu
    bu.upload_artifacts = lambda d: str(d)  # no egress from this container


def build_quant():
    snap = _ensure_snap_op()
    nc = bacc.Bacc(None)
    xin = nc.declare_dram_parameter("xin", [4, 128, QCOLS], F32, isOutput=False)
    win = nc.declare_dram_parameter("w", [C, C, 3, 3], F32, isOutput=False)
    qx = nc.declare_dram_parameter("qx", [4, 128, QCOLS], BF16, isOutput=True)
    qw = nc.declare_dram_parameter("qw", [128, 288], BF16, isOutput=True)

    def bfp(pool, spool, src_ap, ngroups, out_tile):
        """Quantize src_ap [128, ngroups*36] -> out_tile (bf16)."""
        g3 = lambda ap: ap.rearrange("p (g s) -> p g s", s=GS)
        m = spool.tile([128, ngroups], F32, tag="m")
        nc.vector.tensor_reduce(m[:], g3(src_ap), axis=mybir.AxisListType.X,
                                op=mybir.AluOpType.max, apply_absolute_value=True)
        mi = spool.tile([128, ngroups], I32, tag="mi")
        nc.vector.tensor_scalar(mi[:], m[:].bitcast(I32), 0x7F800000, None,
                                op0=mybir.AluOpType.bitwise_and)
        mf = spool.tile([128, ngroups], F32, tag="mf")
        nc.vector.tensor_scalar(mf[:], mi[:].bitcast(F32), MAGIC_MUL, None,
                                op0=mybir.AluOpType.mult)
        mb = mf[:].unsqueeze(-1).broadcast_to([128, ngroups, GS])
        nc.vector._custom_dve(snap, out=g3(out_tile[:]), in0=g3(src_ap), in1=mb)

    with tile.TileContext(nc) as tc:
        with ExitStack() as ctx:
            pool = ctx.enter_context(tc.tile_pool(name="big", bufs=2))
            spool = ctx.enter_context(tc.tile_pool(name="small", bufs=2))
            # weight first: its tiny DMA lands long before sample 0's 3.2MB
            wf = pool.tile([128, 288], F32, tag="wf")
            nc.sync.dma_start(wf[:], win[:].rearrange("o i h w -> (o i h w)")
                              .rearrange("(p c) -> p c", p=128))
            qwt = pool.tile([128, 288], BF16, tag="qwt")
            bfp(pool, spool, wf[:], 8, qwt)
            nc.scalar.dma_start(qw[:], qwt[:])
            xr = xin[:].rearrange("j p c -> p j c")
            qr = qx[:].rearrange("j p c -> p j c")
            for j in range(0, 4, 2):
                xs = pool.tile([128, 2 * QCOLS], F32, tag="xs")
                nc.sync.dma_start(
                    xs[:].rearrange("p (j c) -> p j c", j=2), xr[:, j:j + 2, :])
                q = pool.tile([128, 2 * QCOLS], BF16, tag="q")
                bfp(pool, spool, xs[:], 2 * GPP, q)
                nc.scalar.dma_start(
                    qr[:, j:j + 2, :], q[:].rearrange("p (j c) -> p j c", j=2))
    nc.compile()
    return nc


def build_conv():
    nc = bacc.Bacc(None)
    qx4 = nc.declare_dram_parameter("qx4", [4, C, WP, WP], BF16, isOutput=False)
    wblk = nc.declare_dram_parameter("wblk", [128, 9 * 128], BF16, isOutput=False)
    bias2 = nc.declare_dram_parameter("bias2", [128], F32, isOutput=False)
    out = nc.declare_dram_parameter("out", [4, C, H, W], F32, isOutput=True)

    with tile.TileContext(nc) as tc:
        with ExitStack() as ctx:
            consts = ctx.enter_context(tc.tile_pool(name="consts", bufs=1))
            xpool = ctx.enter_context(tc.tile_pool(name="x", bufs=2))
            opool = ctx.enter_context(tc.tile_pool(name="o", bufs=4))
            psum = ctx.enter_context(tc.tile_pool(name="ps", bufs=4, space="PSUM"))

            # block-diag lhsT per tap: [[W_t, 0], [0, W_t]] so one K=128,M=128
            # matmul convolves both samples of a pair (A on partitions 0-63,
            # B on 64-127) in a single standard accumulation group.
            # Layout built host-side.
            wsb = consts.tile([128, 9 * 128], BF16)
            nc.sync.dma_start(wsb[:], wblk[:])
            bias_sb = consts.tile([128, 1], F32)
            nc.sync.dma_start(bias_sb[:], bias2[:, None])

            out_sc = out[:].rearrange("s c h w -> (s c) h w")

            for p in range(2):
                xpad = xpool.tile([128, XPAD], BF16, tag="xpad")
                nc.gpsimd.memset(xpad[:, 0:1], 0.0)           # guard slots
                nc.gpsimd.memset(xpad[:, XPAD - 1:XPAD], 0.0)
                # host pre-pads qx4 to [C, 114, 114] -> contiguous loads
                nc.sync.dma_start(
                    xpad[0:64, 1:1 + WP * WP],
                    qx4[2 * p].rearrange("c h w -> c (h w)"))
                nc.sync.dma_start(
                    xpad[64:128, 1:1 + WP * WP],
                    qx4[2 * p + 1].rearrange("c h w -> c (h w)"))

                for t in range(14):
                    r0 = 8 * t
                    # two banks per psum tile (bank-aligned halves): rows
                    # r0..r0+3 at cols 0:456, rows r0+4..r0+7 at 512:968;
                    # one strided evac op covers both
                    ps = psum.tile([128, 1024], F32, tag="ps")
                    for half in range(2):
                        rh = r0 + 4 * half
                        for tap in range(9):
                            dh, dw = divmod(tap, 3)
                            base = 1 + (rh + dh) * WP + dw - 1
                            nc.tensor.matmul(
                                ps[:, 512 * half:512 * half + 456],
                                wsb[:, tap * 128:(tap + 1) * 128],
                                xpad[:, base:base + 456],
                                start=(tap == 0), stop=(tap == 8))
                    osb = opool.tile([128, 912], F32, tag="osb")
                    nc.vector.tensor_scalar(
                        osb[:].rearrange("p (h c) -> p h c", h=2),
                        ps[:].rearrange("p (h c) -> p h c", h=2, c=512)[:, :, 0:456],
                        bias_sb[:, 0:1], None,
                        op0=mybir.AluOpType.add)
                    nc.scalar.dma_start(
                        out_sc[2 * p * 64:2 * p * 64 + 128, r0:r0 + 8, :],
                        osb[:].rearrange("p (r w) -> p r w", w=WP)[:, :, 1:113])
    nc.compile()
    return nc


def _shard_inputs(x, weight):
    """Build per-core in_maps for program A."""
    xf = np.ascontiguousarray(x, dtype=np.float32).reshape(-1)
    xf = np.concatenate([xf, np.zeros(QWIN, np.float32)])
    in_maps = []
    pres = []
    for k in range(N_CORES):
        core_pre = []
        xin = np.empty((4, 128, QCOLS), np.float32)
        for j in range(4):
            s = 4 * k + j
            start = s * SAMPLE
            gstart = (start // GS) * GS
            core_pre.append(start - gstart)
            xin[j] = xf[gstart:gstart + QWIN].reshape(128, QCOLS)
        in_maps.append({"xin": xin, "w": np.ascontiguousarray(weight, np.float32)})
        pres.append(core_pre)
    return in_maps, pres


def kernel(x, weight, bias):
    from concourse.bass_utils import run_bass_kernel_spmd

    if "quant" not in _cache:
        _cache["quant"] = build_quant()
    if "conv" not in _cache:
        _cache["conv"] = build_conv()

    core_ids = list(range(N_CORES))
    trace = _trace_enabled()
    if trace:
        _install_trace_shim()

    in_maps, pres = _shard_inputs(x, weight)
    resA = run_bass_kernel_spmd(_cache["quant"], in_maps, core_ids, trace=trace)
    last_exec_ns["quant"] = resA.exec_time_ns
    last_results["quant"] = resA

    bias2 = np.concatenate([np.asarray(bias, np.float32)] * 2)
    in_maps_b = []
    for k in range(N_CORES):
        qx = np.asarray(resA.results[k]["qx"])          # [4,128,QCOLS] bf16
        qw = np.asarray(resA.results[k]["qw"]).reshape(64, 64, 9)  # [o,i,t]
        qx4 = np.zeros((4, C, WP, WP), ml_dtypes.bfloat16)
        for j in range(4):
            pre = pres[k][j]
            qx4[j, :, 1:113, 1:113] = (
                qx[j].reshape(-1)[pre:pre + SAMPLE].reshape(C, H, W))
        wblk = np.zeros((128, 9, 128), ml_dtypes.bfloat16)
        wtio = qw.transpose(1, 2, 0)                    # [i,t,o]
        wblk[0:64, :, 0:64] = wtio
        wblk[64:128, :, 64:128] = wtio
        in_maps_b.append({"qx4": qx4, "wblk": wblk.reshape(128, 9 * 128),
                          "bias2": bias2})
    resB = run_bass_kernel_spmd(_cache["conv"], in_maps_b, core_ids, trace=trace)
    last_exec_ns["conv"] = resB.exec_time_ns
    last_results["conv"] = resB

    out = np.concatenate(
        [np.asarray(resB.results[k]["out"]) for k in range(N_CORES)], axis=0)
    return out.astype(np.float32)



# revision 6
# speedup vs baseline: 1.1259x; 1.1259x over previous
"""BFP-quantized 3x3 conv (nn_BFConv2d) on 8 TRN2 NeuronCores.

Strategy (data-parallel over batch, 4 samples/core):
  Program A (quantize): per core, for each of its 4 samples, load a
    group-aligned window of the flattened x (the BFP group grid is global
    over the flat tensor; each per-sample window starts on a 36-element
    group boundary, so the in-kernel grid is exact), compute the BFP
    quantization with the magic-number trick
        q = (x + M) - M,  M = 1.5 * 2^23 * scale = exp_bits(absmax) * 98304
    (exact round-half-even onto the group lattice; results are <=9
    significant bits so bf16 is exact), and write q as bf16. The weight
    tensor (36864 elems = exactly 1024 groups) is quantized the same way.
  Host: slice each sample's quantized window by its group-grid phase
    (pre in [0,36)) to get slab-aligned q; pure numpy, no device work.
  Program B (conv): 3x3 conv as 9 shifted 64x64 bf16 matmuls per output
    tile, using TensorE 64x64 array tiling: quadrant (0,0) processes the
    even sample of a pair (SBUF partitions 0-63), quadrant (64,64) the odd
    sample (partitions 64-127), both accumulating into one PSUM bank.
    ScalarE evacuates PSUM with the bias add fused; one full-width DMA
    writes both samples' rows (64*12544 == 802816 makes the pair layout
    contiguous in NCHW).
"""

import os
import sys
from contextlib import ExitStack

import numpy as np

sys.path.insert(0, "/opt/trn_rl_repo")

import ml_dtypes  # noqa: E402
import concourse.bacc as bacc  # noqa: E402
import concourse.mybir as mybir  # noqa: E402
import concourse.tile as tile  # noqa: E402

F32 = mybir.dt.float32
BF16 = mybir.dt.bfloat16
I32 = mybir.dt.int32

N_CORES = 8
B = 32                      # batch
C = 64                      # channels (in == out)
H = W = 112
SAMPLE = C * H * W          # 802816 elems per sample
GS = 36                     # BFP group size
GPP = 175                   # groups per partition in the quantize window
QCOLS = GPP * GS            # 6300
QWIN = 128 * QCOLS          # 806400 elems: covers a sample + phase slack
WP = W + 2                  # padded row width 114
XPAD = WP * WP + 2          # padded sample + 2 guard slots
MAGIC_MUL = 98304.0         # 1.5 * 2^16:  exp2(e) * this == 1.5*2^23*2^(e-7)

_cache = {}
last_exec_ns = {}
last_results = {}


def _ensure_snap_op():
    """Register a custom DVE op BFP_SNAP_ANT: out = (in0 + in1) - in1.

    One streaming pass for the BFP magic-number snap (vs add + subtract as
    two scalar_tensor_tensor passes). The per-NEFF DVE table machinery picks
    it up from dve_ops.OPS; sha is pinned from this environment's lowering.
    """
    import concourse.dve_ops as dops
    if getattr(dops, "_BFP_SNAP_ANT", None) is not None:
        return dops._BFP_SNAP_ANT
    from concourse.dve_spec import Spec, Src0, Src1, lower as spec_lower
    from concourse.dve_uop import DveOpSpec

    def _snap_ref(in0, in1, s0, s1, imm2):
        a = in0.astype(np.float32)
        b = np.broadcast_to(in1.astype(np.float32), in1.shape).reshape(a.shape)
        return (a + b) - b

    spec = Spec(body=(Src0 + Src1) - Src1, reference=_snap_ref)
    op = dops.DveOp("BFP_SNAP_ANT", spec, subdim=False, uops_sha={})
    idx = max(dops._SUB_OPCODE_FOR_NAME.values()) + 1
    assert idx < 0x20
    dops.OPS.append(op)
    dops.CUSTOM_DVE_SPECS["BFP_SNAP_ANT"] = spec
    dops._SUB_OPCODE_FOR_NAME["BFP_SNAP_ANT"] = idx
    for ver in ("v3", "v4"):
        try:
            s = DveOpSpec(name=op.name, opcode=idx,
                          uops=spec_lower(spec, ver=ver), rd1_en=True)
            op.uops_sha[ver] = s.sha(ver)
        except Exception:
            pass
    dops._BFP_SNAP_ANT = op
    return op


def _trace_enabled():
    return os.environ.get("BFP_TRACE") == "1"


def _install_trace_shim():
    """Provide antenv.axon_hooks (NTFF profiling hook) if the image lacks it.

    Mirrors trn_agent_boot.trn_boot._ntff_profile_via_ctypes: drives NRT
    profiling through the axon PJRT .so so run_bass_kernel_spmd(trace=True)
    can report HW exec time.
    """
    import types
    import ctypes
    import contextlib
    try:
        from antenv.axon_hooks import get_axon_ntff_profile_hook  # noqa: F401
        return
    except ImportError:
        pass
    so_path = "/opt/axon/libaxon_pjrt.so"
    if not os.path.exists(so_path):
        return
    lib = ctypes.CDLL(so_path)
    if not hasattr(lib, "axon_start_nrt_profile"):
        return
    lib.axon_start_nrt_profile.argtypes = [ctypes.POINTER(ctypes.c_int64),
                                           ctypes.c_size_t]
    lib.axon_start_nrt_profile.restype = ctypes.c_int64
    lib.axon_stop_nrt_profile.argtypes = [ctypes.c_char_p]
    lib.axon_stop_nrt_profile.restype = ctypes.c_int64

    @contextlib.contextmanager
    def _hook(output_dir, device_ids):
        import jax
        jax.devices()
        if device_ids:
            ids = (ctypes.c_int64 * len(device_ids))(*device_ids)
            rc = lib.axon_start_nrt_profile(ids, len(device_ids))
        else:
            rc = lib.axon_start_nrt_profile(None, 0)
        if rc != 0:
            raise RuntimeError(f"axon_start_nrt_profile rc={rc}")
        try:
            yield
        finally:
            n = lib.axon_stop_nrt_profile(str(output_dir).encode())
            print(f"profile: {n} ntff file(s) -> {output_dir}", file=sys.stderr)

    mod = types.ModuleType("antenv.axon_hooks")
    state = {"hook": _hook}
    mod.get_axon_ntff_profile_hook = lambda: state["hook"]
    mod.set_axon_ntff_profile_hook = lambda h: state.update(hook=h)
    sys.modules["antenv.axon_hooks"] = mod
    import antenv
    antenv.axon_hooks = mod
    from concourse import bass_utils as bu
    bu.upload_artifacts = lambda d: str(d)  # no egress from this container


I16 = mybir.dt.int16
# free-dim split of a sample window into 2 group-aligned chunks
CHUNK_COLS = (3168, 3132)          # 88 + 87 groups = 175
CHUNK_OFF = (0, 3168)


def build_quant():
    """v2: scalar casts f32->bf16; vector reduces bf16 at 2x and snaps;
    gpsimd computes per-group magic constants. Half-sample chunks."""
    snap = _ensure_snap_op()
    nc = bacc.Bacc(None)
    xin = nc.declare_dram_parameter("xin", [4, 128, QCOLS], F32, isOutput=False)
    win = nc.declare_dram_parameter("w", [C, C, 3, 3], F32, isOutput=False)
    qx = nc.declare_dram_parameter("qx", [4, 128, QCOLS], BF16, isOutput=True)
    qw = nc.declare_dram_parameter("qw", [128, 288], BF16, isOutput=True)

    def bfp_w(pool, spool, src_ap, ngroups, out_tile):
        """f32 path for the small weight tensor (exact, as v1)."""
        g3 = lambda ap: ap.rearrange("p (g s) -> p g s", s=GS)
        m = spool.tile([128, ngroups], F32, tag="wm")
        nc.vector.tensor_reduce(m[:], g3(src_ap), axis=mybir.AxisListType.X,
                                op=mybir.AluOpType.max, apply_absolute_value=True)
        mi = spool.tile([128, ngroups], I32, tag="wmi")
        nc.vector.tensor_scalar(mi[:], m[:].bitcast(I32), 0x7F800000, None,
                                op0=mybir.AluOpType.bitwise_and)
        mf = spool.tile([128, ngroups], F32, tag="wmf")
        nc.vector.tensor_scalar(mf[:], mi[:].bitcast(F32), MAGIC_MUL, None,
                                op0=mybir.AluOpType.mult)
        mb = mf[:].unsqueeze(-1).broadcast_to([128, ngroups, GS])
        nc.vector._custom_dve(snap, out=g3(out_tile[:]), in0=g3(src_ap), in1=mb)

    with tile.TileContext(nc) as tc:
        with ExitStack() as ctx:
            pool = ctx.enter_context(tc.tile_pool(name="big", bufs=3))
            spool = ctx.enter_context(tc.tile_pool(name="small", bufs=3))
            # weight first: its tiny DMA lands long before sample 0's slab
            wf = pool.tile([128, 288], F32, tag="wf")
            nc.sync.dma_start(wf[:], win[:].rearrange("o i h w -> (o i h w)")
                              .rearrange("(p c) -> p c", p=128))
            qwt = pool.tile([128, 288], BF16, tag="qwt")
            bfp_w(pool, spool, wf[:], 8, qwt)
            nc.scalar.dma_start(qw[:], qwt[:])
            xr = xin[:].rearrange("j p c -> p j c")
            qr = qx[:].rearrange("j p c -> p j c")
            for j in range(4):
                for h in range(2):
                    cols, off = CHUNK_COLS[h], CHUNK_OFF[h]
                    ng = cols // GS
                    g3 = lambda ap: ap.rearrange("p (g s) -> p g s", s=GS)
                    xs = pool.tile([128, cols], F32, tag=f"xs{h}")
                    nc.sync.dma_start(xs[:], xr[:, j, off:off + cols])
                    # scalar engine: cast to bf16 (feeds both reduce and snap)
                    xb = pool.tile([128, cols], BF16, tag=f"xb{h}")
                    nc.scalar.copy(xb[:], xs[:])
                    # vector: group abs-max on bf16 (2x mode)
                    m = spool.tile([128, ng], BF16, tag=f"m{h}")
                    nc.vector.tensor_reduce(
                        m[:], g3(xb[:]), axis=mybir.AxisListType.X,
                        op=mybir.AluOpType.max, apply_absolute_value=True)
                    # M = exp_bits(max) * 1.5*2^16  (tiny)
                    mi = spool.tile([128, ng], I16, tag=f"mi{h}")
                    nc.vector.tensor_scalar(
                        mi[:], m[:].bitcast(I16), 0x7F80, None,
                        op0=mybir.AluOpType.bitwise_and)
                    mf = spool.tile([128, ng], BF16, tag=f"mf{h}")
                    nc.vector.tensor_scalar(
                        mf[:], mi[:].bitcast(BF16), MAGIC_MUL, None,
                        op0=mybir.AluOpType.mult)
                    # vector: snap (x + M) - M in one custom-DVE pass
                    q = pool.tile([128, cols], BF16, tag=f"q{h}")
                    mb = mf[:].unsqueeze(-1).broadcast_to([128, ng, GS])
                    nc.vector._custom_dve(snap, out=g3(q[:]), in0=g3(xb[:]),
                                          in1=mb)
                    nc.scalar.dma_start(qr[:, j, off:off + cols], q[:])
    nc.compile()
    return nc


def build_conv():
    """v2: 4x 64x64 PE-array tiles. T0/T2 convolve sample A (row-chunks 2t,
    2t+1), T8/T10 sample B, all four concurrently; weights replicated on
    both SBUF partition halves. Two PSUM banks per round (A and B)."""
    nc = bacc.Bacc(None)
    qx4 = nc.declare_dram_parameter("qx4", [4, C, WP, WP], BF16, isOutput=False)
    wblk = nc.declare_dram_parameter("wblk", [128, 9 * 64], BF16, isOutput=False)
    bias2 = nc.declare_dram_parameter("bias2", [128], F32, isOutput=False)
    out = nc.declare_dram_parameter("out", [4, C, H, W], F32, isOutput=True)

    with tile.TileContext(nc) as tc:
        with ExitStack() as ctx:
            consts = ctx.enter_context(tc.tile_pool(name="consts", bufs=1))
            xpool = ctx.enter_context(tc.tile_pool(name="x", bufs=2))
            opool = ctx.enter_context(tc.tile_pool(name="o", bufs=4))
            psum = ctx.enter_context(tc.tile_pool(name="ps", bufs=2, space="PSUM"))

            # wblk[p, tap*64 + o]: W[tap][ic=p%64, oc=o], same on both halves
            wsb = consts.tile([128, 9 * 64], BF16)
            nc.sync.dma_start(wsb[:], wblk[:])
            bias_sb = consts.tile([128, 1], F32)
            nc.sync.dma_start(bias_sb[:], bias2[:, None])

            out_s = out[:].rearrange("s c h w -> s c h w")

            for p in range(2):
                xpad = xpool.tile([128, XPAD], BF16, tag="xpad")
                nc.gpsimd.memset(xpad[:, 0:1], 0.0)           # guard slots
                nc.gpsimd.memset(xpad[:, XPAD - 1:XPAD], 0.0)
                # load in 4 row-bands so round 0 can start after band 0;
                # band k covers padded rows [29k, 29k+29) (+1 trailing row)
                for k in range(4):
                    rlo = 29 * k
                    nrows = 29 if k < 3 else 27
                    nc.sync.dma_start(
                        xpad[0:64, 1 + rlo * WP:1 + (rlo + nrows) * WP],
                        qx4[2 * p, :, rlo:rlo + nrows, :]
                        .rearrange("c h w -> c (h w)"))
                    nc.sync.dma_start(
                        xpad[64:128, 1 + rlo * WP:1 + (rlo + nrows) * WP],
                        qx4[2 * p + 1, :, rlo:rlo + nrows, :]
                        .rearrange("c h w -> c (h w)"))

                for t in range(14):
                    r0 = 8 * t
                    psA = psum.tile([128, 456], F32, tag="psA")
                    psB = psum.tile([128, 456], F32, tag="psB")
                    for tap in range(9):
                        dh, dw = divmod(tap, 3)
                        b0 = 1 + (r0 + dh) * WP + dw - 1
                        b1 = 1 + (r0 + 4 + dh) * WP + dw - 1
                        st, sp = (tap == 0), (tap == 8)
                        w_lo = wsb[0:64, tap * 64:(tap + 1) * 64]
                        w_hi = wsb[64:128, tap * 64:(tap + 1) * 64]
                        nc.tensor.matmul(psA[0:64, :], w_lo,
                                         xpad[0:64, b0:b0 + 456],
                                         start=st, stop=sp,
                                         tile_position=(0, 0))
                        nc.tensor.matmul(psB[0:64, :], w_hi,
                                         xpad[64:128, b0:b0 + 456],
                                         start=st, stop=sp,
                                         tile_position=(64, 0))
                        nc.tensor.matmul(psA[64:128, :], w_lo,
                                         xpad[0:64, b1:b1 + 456],
                                         start=st, stop=sp,
                                         tile_position=(0, 64))
                        nc.tensor.matmul(psB[64:128, :], w_hi,
                                         xpad[64:128, b1:b1 + 456],
                                         start=st, stop=sp,
                                         tile_position=(64, 64))
                    for s, ps in ((0, psA), (1, psB)):
                        osb = opool.tile([128, 456], F32, tag=f"osb{s}")
                        nc.vector.tensor_scalar(osb[:], ps[:],
                                                bias_sb[:, 0:1], None,
                                                op0=mybir.AluOpType.add)
                        for u in range(2):
                            nc.scalar.dma_start(
                                out_s[2 * p + s, :, r0 + 4 * u:r0 + 4 * u + 4, :],
                                osb[64 * u:64 * u + 64]
                                .rearrange("p (r w) -> p r w", w=WP)
                                [:, :, 1:113])
    nc.compile()
    return nc


def _shard_inputs(x, weight):
    """Build per-core in_maps for program A."""
    xf = np.ascontiguousarray(x, dtype=np.float32).reshape(-1)
    xf = np.concatenate([xf, np.zeros(QWIN, np.float32)])
    in_maps = []
    pres = []
    for k in range(N_CORES):
        core_pre = []
        xin = np.empty((4, 128, QCOLS), np.float32)
        for j in range(4):
            s = 4 * k + j
            start = s * SAMPLE
            gstart = (start // GS) * GS
            core_pre.append(start - gstart)
            xin[j] = xf[gstart:gstart + QWIN].reshape(128, QCOLS)
        in_maps.append({"xin": xin, "w": np.ascontiguousarray(weight, np.float32)})
        pres.append(core_pre)
    return in_maps, pres


def kernel(x, weight, bias):
    from concourse.bass_utils import run_bass_kernel_spmd

    if "quant" not in _cache:
        _cache["quant"] = build_quant()
    if "conv" not in _cache:
        _cache["conv"] = build_conv()

    core_ids = list(range(N_CORES))
    trace = _trace_enabled()
    if trace:
        _install_trace_shim()

    in_maps, pres = _shard_inputs(x, weight)
    resA = run_bass_kernel_spmd(_cache["quant"], in_maps, core_ids, trace=trace)
    last_exec_ns["quant"] = resA.exec_time_ns
    last_results["quant"] = resA

    bias2 = np.concatenate([np.asarray(bias, np.float32)] * 2)
    in_maps_b = []
    for k in range(N_CORES):
        qx = np.asarray(resA.results[k]["qx"])          # [4,128,QCOLS] bf16
        qw = np.asarray(resA.results[k]["qw"]).reshape(64, 64, 9)  # [o,i,t]
        qx4 = np.zeros((4, C, WP, WP), ml_dtypes.bfloat16)
        for j in range(4):
            pre = pres[k][j]
            qx4[j, :, 1:113, 1:113] = (
                qx[j].reshape(-1)[pre:pre + SAMPLE].reshape(C, H, W))
        wtio = qw.transpose(1, 2, 0)                    # [i,t,o]
        wblk = np.concatenate([wtio, wtio], axis=0)     # [128,9,64] both halves
        in_maps_b.append({"qx4": qx4, "wblk": wblk.reshape(128, 9 * 64),
                          "bias2": bias2})
    resB = run_bass_kernel_spmd(_cache["conv"], in_maps_b, core_ids, trace=trace)
    last_exec_ns["conv"] = resB.exec_time_ns
    last_results["conv"] = resB

    out = np.concatenate(
        [np.asarray(resB.results[k]["out"]) for k in range(N_CORES)], axis=0)
    return out.astype(np.float32)



# revision 10
# speedup vs baseline: 1.2622x; 1.1211x over previous
"""BFP-quantized 3x3 conv (nn_BFConv2d) on 8 TRN2 NeuronCores.

Strategy (data-parallel over batch, 4 samples/core):
  Program A (quantize): per core, for each of its 4 samples, load a
    group-aligned window of the flattened x (the BFP group grid is global
    over the flat tensor; each per-sample window starts on a 36-element
    group boundary, so the in-kernel grid is exact), compute the BFP
    quantization with the magic-number trick
        q = (x + M) - M,  M = 1.5 * 2^23 * scale = exp_bits(absmax) * 98304
    (exact round-half-even onto the group lattice; results are <=9
    significant bits so bf16 is exact), and write q as bf16. The weight
    tensor (36864 elems = exactly 1024 groups) is quantized the same way.
  Host: slice each sample's quantized window by its group-grid phase
    (pre in [0,36)) to get slab-aligned q; pure numpy, no device work.
  Program B (conv): 3x3 conv as 9 shifted 64x64 bf16 matmuls per output
    tile, using TensorE 64x64 array tiling: quadrant (0,0) processes the
    even sample of a pair (SBUF partitions 0-63), quadrant (64,64) the odd
    sample (partitions 64-127), both accumulating into one PSUM bank.
    ScalarE evacuates PSUM with the bias add fused; one full-width DMA
    writes both samples' rows (64*12544 == 802816 makes the pair layout
    contiguous in NCHW).
"""

import os
import sys
from contextlib import ExitStack

import numpy as np

sys.path.insert(0, "/opt/trn_rl_repo")

import ml_dtypes  # noqa: E402
import concourse.bacc as bacc  # noqa: E402
import concourse.mybir as mybir  # noqa: E402
import concourse.tile as tile  # noqa: E402

F32 = mybir.dt.float32
BF16 = mybir.dt.bfloat16
I32 = mybir.dt.int32

N_CORES = 8
B = 32                      # batch
C = 64                      # channels (in == out)
H = W = 112
SAMPLE = C * H * W          # 802816 elems per sample
GS = 36                     # BFP group size
GPP = 175                   # groups per partition in the quantize window
QCOLS = GPP * GS            # 6300
QWIN = 128 * QCOLS          # 806400 elems: covers a sample + phase slack
WP = W + 2                  # padded row width 114
XPAD = WP * WP + 2          # padded sample + 2 guard slots
MAGIC_MUL = 98304.0         # 1.5 * 2^16:  exp2(e) * this == 1.5*2^23*2^(e-7)

_cache = {}
last_exec_ns = {}
last_results = {}


def _ensure_snap_op():
    """Register a custom DVE op BFP_SNAP_ANT: out = (in0 + in1) - in1.

    One streaming pass for the BFP magic-number snap (vs add + subtract as
    two scalar_tensor_tensor passes). The per-NEFF DVE table machinery picks
    it up from dve_ops.OPS; sha is pinned from this environment's lowering.
    """
    import concourse.dve_ops as dops
    if getattr(dops, "_BFP_SNAP_ANT", None) is not None:
        return dops._BFP_SNAP_ANT
    from concourse.dve_spec import Spec, Src0, Src1, lower as spec_lower
    from concourse.dve_uop import DveOpSpec

    def _snap_ref(in0, in1, s0, s1, imm2):
        a = in0.astype(np.float32)
        b = np.broadcast_to(in1.astype(np.float32), in1.shape).reshape(a.shape)
        return (a + b) - b

    spec = Spec(body=(Src0 + Src1) - Src1, reference=_snap_ref)
    op = dops.DveOp("BFP_SNAP_ANT", spec, subdim=False, uops_sha={})
    idx = max(dops._SUB_OPCODE_FOR_NAME.values()) + 1
    assert idx < 0x20
    dops.OPS.append(op)
    dops.CUSTOM_DVE_SPECS["BFP_SNAP_ANT"] = spec
    dops._SUB_OPCODE_FOR_NAME["BFP_SNAP_ANT"] = idx
    for ver in ("v3", "v4"):
        try:
            s = DveOpSpec(name=op.name, opcode=idx,
                          uops=spec_lower(spec, ver=ver), rd1_en=True)
            op.uops_sha[ver] = s.sha(ver)
        except Exception:
            pass
    dops._BFP_SNAP_ANT = op
    return op


def _trace_enabled():
    return os.environ.get("BFP_TRACE") == "1"


def _install_trace_shim():
    """Provide antenv.axon_hooks (NTFF profiling hook) if the image lacks it.

    Mirrors trn_agent_boot.trn_boot._ntff_profile_via_ctypes: drives NRT
    profiling through the axon PJRT .so so run_bass_kernel_spmd(trace=True)
    can report HW exec time.
    """
    import types
    import ctypes
    import contextlib
    try:
        from antenv.axon_hooks import get_axon_ntff_profile_hook  # noqa: F401
        return
    except ImportError:
        pass
    so_path = "/opt/axon/libaxon_pjrt.so"
    if not os.path.exists(so_path):
        return
    lib = ctypes.CDLL(so_path)
    if not hasattr(lib, "axon_start_nrt_profile"):
        return
    lib.axon_start_nrt_profile.argtypes = [ctypes.POINTER(ctypes.c_int64),
                                           ctypes.c_size_t]
    lib.axon_start_nrt_profile.restype = ctypes.c_int64
    lib.axon_stop_nrt_profile.argtypes = [ctypes.c_char_p]
    lib.axon_stop_nrt_profile.restype = ctypes.c_int64

    @contextlib.contextmanager
    def _hook(output_dir, device_ids):
        import jax
        jax.devices()
        if device_ids:
            ids = (ctypes.c_int64 * len(device_ids))(*device_ids)
            rc = lib.axon_start_nrt_profile(ids, len(device_ids))
        else:
            rc = lib.axon_start_nrt_profile(None, 0)
        if rc != 0:
            raise RuntimeError(f"axon_start_nrt_profile rc={rc}")
        try:
            yield
        finally:
            n = lib.axon_stop_nrt_profile(str(output_dir).encode())
            print(f"profile: {n} ntff file(s) -> {output_dir}", file=sys.stderr)

    mod = types.ModuleType("antenv.axon_hooks")
    state = {"hook": _hook}
    mod.get_axon_ntff_profile_hook = lambda: state["hook"]
    mod.set_axon_ntff_profile_hook = lambda h: state.update(hook=h)
    sys.modules["antenv.axon_hooks"] = mod
    import antenv
    antenv.axon_hooks = mod
    from concourse import bass_utils as bu
    bu.upload_artifacts = lambda d: str(d)  # no egress from this container


I16 = mybir.dt.int16
# free-dim split of a sample window into 2 group-aligned chunks
CHUNK_COLS = (3168, 3132)          # 88 + 87 groups = 175
CHUNK_OFF = (0, 3168)


def build_quant():
    """v2: scalar casts f32->bf16; vector reduces bf16 at 2x and snaps;
    gpsimd computes per-group magic constants. Half-sample chunks."""
    snap = _ensure_snap_op()
    nc = bacc.Bacc(None)
    xin = nc.declare_dram_parameter("xin", [4, 128, QCOLS], F32, isOutput=False)
    win = nc.declare_dram_parameter("w", [C, C, 3, 3], F32, isOutput=False)
    qx = nc.declare_dram_parameter("qx", [4, 128, QCOLS], BF16, isOutput=True)
    qw = nc.declare_dram_parameter("qw", [128, 288], BF16, isOutput=True)

    def bfp_w(pool, spool, src_ap, ngroups, out_tile):
        """f32 path for the small weight tensor (exact, as v1)."""
        g3 = lambda ap: ap.rearrange("p (g s) -> p g s", s=GS)
        m = spool.tile([128, ngroups], F32, tag="wm")
        nc.vector.tensor_reduce(m[:], g3(src_ap), axis=mybir.AxisListType.X,
                                op=mybir.AluOpType.max, apply_absolute_value=True)
        mi = spool.tile([128, ngroups], I32, tag="wmi")
        nc.vector.tensor_scalar(mi[:], m[:].bitcast(I32), 0x7F800000, None,
                                op0=mybir.AluOpType.bitwise_and)
        mf = spool.tile([128, ngroups], F32, tag="wmf")
        nc.vector.tensor_scalar(mf[:], mi[:].bitcast(F32), MAGIC_MUL, None,
                                op0=mybir.AluOpType.mult)
        mb = mf[:].unsqueeze(-1).broadcast_to([128, ngroups, GS])
        nc.vector._custom_dve(snap, out=g3(out_tile[:]), in0=g3(src_ap), in1=mb)

    with tile.TileContext(nc) as tc:
        with ExitStack() as ctx:
            pool = ctx.enter_context(tc.tile_pool(name="big", bufs=3))
            spool = ctx.enter_context(tc.tile_pool(name="small", bufs=3))
            # weight first: its tiny DMA lands long before sample 0's slab
            wf = pool.tile([128, 288], F32, tag="wf")
            nc.sync.dma_start(wf[:], win[:].rearrange("o i h w -> (o i h w)")
                              .rearrange("(p c) -> p c", p=128))
            qwt = pool.tile([128, 288], BF16, tag="qwt")
            bfp_w(pool, spool, wf[:], 8, qwt)
            nc.scalar.dma_start(qw[:], qwt[:])
            xr = xin[:].rearrange("j p c -> p j c")
            qr = qx[:].rearrange("j p c -> p j c")
            for j in range(4):
                for h in range(2):
                    cols, off = CHUNK_COLS[h], CHUNK_OFF[h]
                    ng = cols // GS
                    g3 = lambda ap: ap.rearrange("p (g s) -> p g s", s=GS)
                    xs = pool.tile([128, cols], F32, tag=f"xs{h}")
                    nc.sync.dma_start(xs[:], xr[:, j, off:off + cols])
                    # scalar engine: cast to bf16 (feeds both reduce and snap)
                    xb = pool.tile([128, cols], BF16, tag=f"xb{h}")
                    nc.scalar.copy(xb[:], xs[:])
                    # vector: group abs-max on bf16 (2x mode)
                    m = spool.tile([128, ng], BF16, tag=f"m{h}")
                    nc.vector.tensor_reduce(
                        m[:], g3(xb[:]), axis=mybir.AxisListType.X,
                        op=mybir.AluOpType.max, apply_absolute_value=True)
                    # M = exp_bits(max) * 1.5*2^16  (tiny)
                    mi = spool.tile([128, ng], I16, tag=f"mi{h}")
                    nc.vector.tensor_scalar(
                        mi[:], m[:].bitcast(I16), 0x7F80, None,
                        op0=mybir.AluOpType.bitwise_and)
                    mf = spool.tile([128, ng], BF16, tag=f"mf{h}")
                    nc.vector.tensor_scalar(
                        mf[:], mi[:].bitcast(BF16), MAGIC_MUL, None,
                        op0=mybir.AluOpType.mult)
                    # vector: snap (x + M) - M in one custom-DVE pass
                    q = pool.tile([128, cols], BF16, tag=f"q{h}")
                    mb = mf[:].unsqueeze(-1).broadcast_to([128, ng, GS])
                    nc.vector._custom_dve(snap, out=g3(q[:]), in0=g3(xb[:]),
                                          in1=mb)
                    nc.scalar.dma_start(qr[:, j, off:off + cols], q[:])
    nc.compile()
    return nc


def build_conv():
    """v2: 4x 64x64 PE-array tiles. T0/T2 convolve sample A (row-chunks 2t,
    2t+1), T8/T10 sample B, all four concurrently; weights replicated on
    both SBUF partition halves. Two PSUM banks per round (A and B)."""
    nc = bacc.Bacc(None)
    qx4 = nc.declare_dram_parameter("qx4", [4, C, WP, WP], BF16, isOutput=False)
    wblk = nc.declare_dram_parameter("wblk", [128, 9 * 64], BF16, isOutput=False)
    bias2 = nc.declare_dram_parameter("bias2", [128], F32, isOutput=False)
    # chunk-major: [sample, 4-row chunk, c, r, w]; host transposes to NCHW.
    # Keeps each out-DMA one call with 128 contiguous 1792B descriptors.
    out = nc.declare_dram_parameter("out", [4, 28, C, 4, W], F32, isOutput=True)

    with tile.TileContext(nc) as tc:
        with ExitStack() as ctx:
            consts = ctx.enter_context(tc.tile_pool(name="consts", bufs=1))
            xpool = ctx.enter_context(tc.tile_pool(name="x", bufs=2))
            opool = ctx.enter_context(tc.tile_pool(name="o", bufs=4))
            psum = ctx.enter_context(tc.tile_pool(name="ps", bufs=2, space="PSUM"))

            # wblk[p, tap*64 + o]: W[tap][ic=p%64, oc=o], same on both halves
            wsb = consts.tile([128, 9 * 64], BF16)
            nc.sync.dma_start(wsb[:], wblk[:])
            bias_sb = consts.tile([128, 1], F32)
            nc.sync.dma_start(bias_sb[:], bias2[:, None])

            for p in range(2):
                xpad = xpool.tile([128, XPAD], BF16, tag="xpad")
                nc.gpsimd.memset(xpad[:, 0:1], 0.0)           # guard slots
                nc.gpsimd.memset(xpad[:, XPAD - 1:XPAD], 0.0)
                # load in 4 row-bands so round 0 can start after band 0;
                # band k covers padded rows [29k, 29k+29) (+1 trailing row)
                for k in range(4):
                    rlo = 29 * k
                    nrows = 29 if k < 3 else 27
                    nc.sync.dma_start(
                        xpad[0:64, 1 + rlo * WP:1 + (rlo + nrows) * WP],
                        qx4[2 * p, :, rlo:rlo + nrows, :]
                        .rearrange("c h w -> c (h w)"))
                    nc.sync.dma_start(
                        xpad[64:128, 1 + rlo * WP:1 + (rlo + nrows) * WP],
                        qx4[2 * p + 1, :, rlo:rlo + nrows, :]
                        .rearrange("c h w -> c (h w)"))

                for t in range(14):
                    r0 = 8 * t
                    psA = psum.tile([128, 456], F32, tag="psA")
                    psB = psum.tile([128, 456], F32, tag="psB")
                    for tap in range(9):
                        dh, dw = divmod(tap, 3)
                        b0 = 1 + (r0 + dh) * WP + dw - 1
                        b1 = 1 + (r0 + 4 + dh) * WP + dw - 1
                        st, sp = (tap == 0), (tap == 8)
                        w_lo = wsb[0:64, tap * 64:(tap + 1) * 64]
                        w_hi = wsb[64:128, tap * 64:(tap + 1) * 64]
                        nc.tensor.matmul(psA[0:64, :], w_lo,
                                         xpad[0:64, b0:b0 + 456],
                                         start=st, stop=sp,
                                         tile_position=(0, 0))
                        nc.tensor.matmul(psB[0:64, :], w_hi,
                                         xpad[64:128, b0:b0 + 456],
                                         start=st, stop=sp,
                                         tile_position=(64, 0))
                        nc.tensor.matmul(psA[64:128, :], w_lo,
                                         xpad[0:64, b1:b1 + 456],
                                         start=st, stop=sp,
                                         tile_position=(0, 64))
                        nc.tensor.matmul(psB[64:128, :], w_hi,
                                         xpad[64:128, b1:b1 + 456],
                                         start=st, stop=sp,
                                         tile_position=(64, 64))
                    for s, ps in ((0, psA), (1, psB)):
                        osb = opool.tile([128, 456], F32, tag=f"osb{s}")
                        nc.vector.tensor_scalar(osb[:], ps[:],
                                                bias_sb[:, 0:1], None,
                                                op0=mybir.AluOpType.add)
                        nc.scalar.dma_start(
                            out[2 * p + s, 2 * t:2 * t + 2]
                            .rearrange("u c r w -> (u c) r w"),
                            osb[:].rearrange("p (r w) -> p r w", w=WP)
                            [:, :, 1:113])
    nc.compile()
    return nc


def _shard_inputs(x, weight):
    """Build per-core in_maps for program A."""
    xf = np.ascontiguousarray(x, dtype=np.float32).reshape(-1)
    xf = np.concatenate([xf, np.zeros(QWIN, np.float32)])
    in_maps = []
    pres = []
    for k in range(N_CORES):
        core_pre = []
        xin = np.empty((4, 128, QCOLS), np.float32)
        for j in range(4):
            s = 4 * k + j
            start = s * SAMPLE
            gstart = (start // GS) * GS
            core_pre.append(start - gstart)
            xin[j] = xf[gstart:gstart + QWIN].reshape(128, QCOLS)
        in_maps.append({"xin": xin, "w": np.ascontiguousarray(weight, np.float32)})
        pres.append(core_pre)
    return in_maps, pres


def kernel(x, weight, bias):
    from concourse.bass_utils import run_bass_kernel_spmd

    if "quant" not in _cache:
        _cache["quant"] = build_quant()
    if "conv" not in _cache:
        _cache["conv"] = build_conv()

    core_ids = list(range(N_CORES))
    trace = _trace_enabled()
    if trace:
        _install_trace_shim()

    in_maps, pres = _shard_inputs(x, weight)
    resA = run_bass_kernel_spmd(_cache["quant"], in_maps, core_ids, trace=trace)
    last_exec_ns["quant"] = resA.exec_time_ns
    last_results["quant"] = resA

    bias2 = np.concatenate([np.asarray(bias, np.float32)] * 2)
    in_maps_b = []
    for k in range(N_CORES):
        qx = np.asarray(resA.results[k]["qx"])          # [4,128,QCOLS] bf16
        qw = np.asarray(resA.results[k]["qw"]).reshape(64, 64, 9)  # [o,i,t]
        qx4 = np.zeros((4, C, WP, WP), ml_dtypes.bfloat16)
        for j in range(4):
            pre = pres[k][j]
            qx4[j, :, 1:113, 1:113] = (
                qx[j].reshape(-1)[pre:pre + SAMPLE].reshape(C, H, W))
        wtio = qw.transpose(1, 2, 0)                    # [i,t,o]
        wblk = np.concatenate([wtio, wtio], axis=0)     # [128,9,64] both halves
        in_maps_b.append({"qx4": qx4, "wblk": wblk.reshape(128, 9 * 64),
                          "bias2": bias2})
    resB = run_bass_kernel_spmd(_cache["conv"], in_maps_b, core_ids, trace=trace)
    last_exec_ns["conv"] = resB.exec_time_ns
    last_results["conv"] = resB

    out = np.concatenate(
        [np.asarray(resB.results[k]["out"]) for k in range(N_CORES)], axis=0)
    # [32, 28, C, 4, W] chunk-major -> NCHW
    out = out.transpose(0, 2, 1, 3, 4).reshape(B, C, H, W)
    return np.ascontiguousarray(out, dtype=np.float32)



# revision 13
# speedup vs baseline: 1.2772x; 1.0118x over previous
"""BFP-quantized 3x3 conv (nn_BFConv2d) on 8 TRN2 NeuronCores.

Strategy (data-parallel over batch, 4 samples/core):
  Program A (quantize): per core, for each of its 4 samples, load a
    group-aligned window of the flattened x (the BFP group grid is global
    over the flat tensor; each per-sample window starts on a 36-element
    group boundary, so the in-kernel grid is exact), compute the BFP
    quantization with the magic-number trick
        q = (x + M) - M,  M = 1.5 * 2^23 * scale = exp_bits(absmax) * 98304
    (exact round-half-even onto the group lattice; results are <=9
    significant bits so bf16 is exact), and write q as bf16. The weight
    tensor (36864 elems = exactly 1024 groups) is quantized the same way.
  Host: slice each sample's quantized window by its group-grid phase
    (pre in [0,36)) to get slab-aligned q; pure numpy, no device work.
  Program B (conv): 3x3 conv as 9 shifted 64x64 bf16 matmuls per output
    tile, using TensorE 64x64 array tiling: quadrant (0,0) processes the
    even sample of a pair (SBUF partitions 0-63), quadrant (64,64) the odd
    sample (partitions 64-127), both accumulating into one PSUM bank.
    ScalarE evacuates PSUM with the bias add fused; one full-width DMA
    writes both samples' rows (64*12544 == 802816 makes the pair layout
    contiguous in NCHW).
"""

import os
import sys
from contextlib import ExitStack

import numpy as np

sys.path.insert(0, "/opt/trn_rl_repo")

import ml_dtypes  # noqa: E402
import concourse.bacc as bacc  # noqa: E402
import concourse.mybir as mybir  # noqa: E402
import concourse.tile as tile  # noqa: E402

F32 = mybir.dt.float32
BF16 = mybir.dt.bfloat16
I32 = mybir.dt.int32

N_CORES = 8
B = 32                      # batch
C = 64                      # channels (in == out)
H = W = 112
SAMPLE = C * H * W          # 802816 elems per sample
GS = 36                     # BFP group size
GPP = 175                   # groups per partition in the quantize window
QCOLS = GPP * GS            # 6300
QWIN = 128 * QCOLS          # 806400 elems: covers a sample + phase slack
WP = W + 2                  # padded row width 114
XPAD = WP * WP + 2          # padded sample + 2 guard slots
MAGIC_MUL = 98304.0         # 1.5 * 2^16:  exp2(e) * this == 1.5*2^23*2^(e-7)

_cache = {}
last_exec_ns = {}
last_results = {}


def _ensure_snap_op():
    """Register a custom DVE op BFP_SNAP_ANT: out = (in0 + in1) - in1.

    One streaming pass for the BFP magic-number snap (vs add + subtract as
    two scalar_tensor_tensor passes). The per-NEFF DVE table machinery picks
    it up from dve_ops.OPS; sha is pinned from this environment's lowering.
    """
    import concourse.dve_ops as dops
    if getattr(dops, "_BFP_SNAP_ANT", None) is not None:
        return dops._BFP_SNAP_ANT
    from concourse.dve_spec import Spec, Src0, Src1, lower as spec_lower
    from concourse.dve_uop import DveOpSpec

    def _snap_ref(in0, in1, s0, s1, imm2):
        a = in0.astype(np.float32)
        b = np.broadcast_to(in1.astype(np.float32), in1.shape).reshape(a.shape)
        return (a + b) - b

    spec = Spec(body=(Src0 + Src1) - Src1, reference=_snap_ref)
    op = dops.DveOp("BFP_SNAP_ANT", spec, subdim=False, uops_sha={})
    idx = max(dops._SUB_OPCODE_FOR_NAME.values()) + 1
    assert idx < 0x20
    dops.OPS.append(op)
    dops.CUSTOM_DVE_SPECS["BFP_SNAP_ANT"] = spec
    dops._SUB_OPCODE_FOR_NAME["BFP_SNAP_ANT"] = idx
    for ver in ("v3", "v4"):
        try:
            s = DveOpSpec(name=op.name, opcode=idx,
                          uops=spec_lower(spec, ver=ver), rd1_en=True)
            op.uops_sha[ver] = s.sha(ver)
        except Exception:
            pass
    dops._BFP_SNAP_ANT = op
    return op


def _trace_enabled():
    return os.environ.get("BFP_TRACE") == "1"


def _install_trace_shim():
    """Provide antenv.axon_hooks (NTFF profiling hook) if the image lacks it.

    Mirrors trn_agent_boot.trn_boot._ntff_profile_via_ctypes: drives NRT
    profiling through the axon PJRT .so so run_bass_kernel_spmd(trace=True)
    can report HW exec time.
    """
    import types
    import ctypes
    import contextlib
    try:
        from antenv.axon_hooks import get_axon_ntff_profile_hook  # noqa: F401
        return
    except ImportError:
        pass
    so_path = "/opt/axon/libaxon_pjrt.so"
    if not os.path.exists(so_path):
        return
    lib = ctypes.CDLL(so_path)
    if not hasattr(lib, "axon_start_nrt_profile"):
        return
    lib.axon_start_nrt_profile.argtypes = [ctypes.POINTER(ctypes.c_int64),
                                           ctypes.c_size_t]
    lib.axon_start_nrt_profile.restype = ctypes.c_int64
    lib.axon_stop_nrt_profile.argtypes = [ctypes.c_char_p]
    lib.axon_stop_nrt_profile.restype = ctypes.c_int64

    @contextlib.contextmanager
    def _hook(output_dir, device_ids):
        import jax
        jax.devices()
        if device_ids:
            ids = (ctypes.c_int64 * len(device_ids))(*device_ids)
            rc = lib.axon_start_nrt_profile(ids, len(device_ids))
        else:
            rc = lib.axon_start_nrt_profile(None, 0)
        if rc != 0:
            raise RuntimeError(f"axon_start_nrt_profile rc={rc}")
        try:
            yield
        finally:
            n = lib.axon_stop_nrt_profile(str(output_dir).encode())
            print(f"profile: {n} ntff file(s) -> {output_dir}", file=sys.stderr)

    mod = types.ModuleType("antenv.axon_hooks")
    state = {"hook": _hook}
    mod.get_axon_ntff_profile_hook = lambda: state["hook"]
    mod.set_axon_ntff_profile_hook = lambda h: state.update(hook=h)
    sys.modules["antenv.axon_hooks"] = mod
    import antenv
    antenv.axon_hooks = mod
    from concourse import bass_utils as bu
    bu.upload_artifacts = lambda d: str(d)  # no egress from this container


I16 = mybir.dt.int16
# free-dim split of a sample window into 2 group-aligned chunks
CHUNK_COLS = (3168, 3132)          # 88 + 87 groups = 175
CHUNK_OFF = (0, 3168)


def build_quant():
    """v2: scalar casts f32->bf16; vector reduces bf16 at 2x and snaps;
    gpsimd computes per-group magic constants. Half-sample chunks."""
    snap = _ensure_snap_op()
    nc = bacc.Bacc(None)
    xin = nc.declare_dram_parameter("xin", [4, 128, QCOLS], F32, isOutput=False)
    win = nc.declare_dram_parameter("w", [C, C, 3, 3], F32, isOutput=False)
    qx = nc.declare_dram_parameter("qx", [4, 128, QCOLS], BF16, isOutput=True)
    qw = nc.declare_dram_parameter("qw", [128, 288], BF16, isOutput=True)

    def bfp_w(pool, spool, src_ap, ngroups, out_tile):
        """f32 path for the small weight tensor (exact, as v1)."""
        g3 = lambda ap: ap.rearrange("p (g s) -> p g s", s=GS)
        m = spool.tile([128, ngroups], F32, tag="wm")
        nc.vector.tensor_reduce(m[:], g3(src_ap), axis=mybir.AxisListType.X,
                                op=mybir.AluOpType.max, apply_absolute_value=True)
        mi = spool.tile([128, ngroups], I32, tag="wmi")
        nc.vector.tensor_scalar(mi[:], m[:].bitcast(I32), 0x7F800000, None,
                                op0=mybir.AluOpType.bitwise_and)
        mf = spool.tile([128, ngroups], F32, tag="wmf")
        nc.vector.tensor_scalar(mf[:], mi[:].bitcast(F32), MAGIC_MUL, None,
                                op0=mybir.AluOpType.mult)
        mb = mf[:].unsqueeze(-1).broadcast_to([128, ngroups, GS])
        nc.vector._custom_dve(snap, out=g3(out_tile[:]), in0=g3(src_ap), in1=mb)

    with tile.TileContext(nc) as tc:
        with ExitStack() as ctx:
            pool = ctx.enter_context(tc.tile_pool(name="big", bufs=3))
            spool = ctx.enter_context(tc.tile_pool(name="small", bufs=3))
            # weight first: its tiny DMA lands long before sample 0's slab
            wf = pool.tile([128, 288], F32, tag="wf")
            nc.sync.dma_start(wf[:], win[:].rearrange("o i h w -> (o i h w)")
                              .rearrange("(p c) -> p c", p=128))
            qwt = pool.tile([128, 288], BF16, tag="qwt")
            bfp_w(pool, spool, wf[:], 8, qwt)
            nc.scalar.dma_start(qw[:], qwt[:])
            xr = xin[:].rearrange("j p c -> p j c")
            qr = qx[:].rearrange("j p c -> p j c")
            for j in range(4):
                for h in range(2):
                    cols, off = CHUNK_COLS[h], CHUNK_OFF[h]
                    ng = cols // GS
                    g3 = lambda ap: ap.rearrange("p (g s) -> p g s", s=GS)
                    xs = pool.tile([128, cols], F32, tag=f"xs{h}")
                    nc.sync.dma_start(xs[:], xr[:, j, off:off + cols])
                    # scalar engine: cast to bf16 (feeds both reduce and snap)
                    xb = pool.tile([128, cols], BF16, tag=f"xb{h}")
                    nc.scalar.copy(xb[:], xs[:])
                    # vector: group abs-max on bf16 (2x mode)
                    m = spool.tile([128, ng], BF16, tag=f"m{h}")
                    nc.vector.tensor_reduce(
                        m[:], g3(xb[:]), axis=mybir.AxisListType.X,
                        op=mybir.AluOpType.max, apply_absolute_value=True)
                    # M = exp_bits(max) * 1.5*2^16  (tiny)
                    mi = spool.tile([128, ng], I16, tag=f"mi{h}")
                    nc.vector.tensor_scalar(
                        mi[:], m[:].bitcast(I16), 0x7F80, None,
                        op0=mybir.AluOpType.bitwise_and)
                    mf = spool.tile([128, ng], BF16, tag=f"mf{h}")
                    nc.vector.tensor_scalar(
                        mf[:], mi[:].bitcast(BF16), MAGIC_MUL, None,
                        op0=mybir.AluOpType.mult)
                    # snap (x + M) - M: vector takes the first VG groups in
                    # one custom-DVE pass; pool does the rest as 2 passes
                    q = pool.tile([128, cols], BF16, tag=f"q{h}")
                    vg = (ng * 11 + 10) // 21          # vector share ~0.52
                    mb = mf[:, 0:vg].unsqueeze(-1).broadcast_to([128, vg, GS])
                    nc.vector._custom_dve(snap, out=g3(q[:, :vg * GS]),
                                          in0=g3(xb[:, :vg * GS]), in1=mb)
                    pg = ng - vg
                    mp = mf[:, vg:].unsqueeze(-1).broadcast_to([128, pg, GS])
                    tpool_t = pool.tile([128, cols - vg * GS], F32,
                                        tag=f"tp{h}")
                    nc.gpsimd.tensor_tensor(
                        g3(tpool_t[:]), g3(xb[:, vg * GS:]), mp,
                        op=mybir.AluOpType.add)
                    nc.gpsimd.tensor_tensor(
                        g3(q[:, vg * GS:]), g3(tpool_t[:]), mp,
                        op=mybir.AluOpType.subtract)
                    nc.scalar.dma_start(qr[:, j, off:off + cols], q[:])
    nc.compile()
    return nc


def build_conv():
    """v2: 4x 64x64 PE-array tiles. T0/T2 convolve sample A (row-chunks 2t,
    2t+1), T8/T10 sample B, all four concurrently; weights replicated on
    both SBUF partition halves. Two PSUM banks per round (A and B)."""
    nc = bacc.Bacc(None)
    qx4 = nc.declare_dram_parameter("qx4", [4, C, WP, WP], BF16, isOutput=False)
    wblk = nc.declare_dram_parameter("wblk", [128, 9 * 64], BF16, isOutput=False)
    bias2 = nc.declare_dram_parameter("bias2", [128], F32, isOutput=False)
    # chunk-major: [sample, 4-row chunk, c, r, w]; host transposes to NCHW.
    # Keeps each out-DMA one call with 128 contiguous 1792B descriptors.
    out = nc.declare_dram_parameter("out", [4, 28, C, 4, W], F32, isOutput=True)

    with tile.TileContext(nc) as tc:
        with ExitStack() as ctx:
            consts = ctx.enter_context(tc.tile_pool(name="consts", bufs=1))
            xpool = ctx.enter_context(tc.tile_pool(name="x", bufs=2))
            opool = ctx.enter_context(tc.tile_pool(name="o", bufs=4))
            psum = ctx.enter_context(tc.tile_pool(name="ps", bufs=2, space="PSUM"))

            # wblk[p, tap*64 + o]: W[tap][ic=p%64, oc=o], same on both halves
            wsb = consts.tile([128, 9 * 64], BF16)
            nc.sync.dma_start(wsb[:], wblk[:])
            bias_sb = consts.tile([128, 1], F32)
            nc.sync.dma_start(bias_sb[:], bias2[:, None])

            for p in range(2):
                xpad = xpool.tile([128, XPAD], BF16, tag="xpad")
                nc.gpsimd.memset(xpad[:, 0:1], 0.0)           # guard slots
                nc.gpsimd.memset(xpad[:, XPAD - 1:XPAD], 0.0)
                # load in 4 row-bands so round 0 can start after band 0;
                # band k covers padded rows [29k, 29k+29) (+1 trailing row)
                for k in range(4):
                    rlo = 29 * k
                    nrows = 29 if k < 3 else 27
                    nc.sync.dma_start(
                        xpad[0:64, 1 + rlo * WP:1 + (rlo + nrows) * WP],
                        qx4[2 * p, :, rlo:rlo + nrows, :]
                        .rearrange("c h w -> c (h w)"))
                    nc.sync.dma_start(
                        xpad[64:128, 1 + rlo * WP:1 + (rlo + nrows) * WP],
                        qx4[2 * p + 1, :, rlo:rlo + nrows, :]
                        .rearrange("c h w -> c (h w)"))

                for t in range(14):
                    r0 = 8 * t
                    psA = psum.tile([128, 456], F32, tag="psA")
                    psB = psum.tile([128, 456], F32, tag="psB")
                    for tap in range(9):
                        dh, dw = divmod(tap, 3)
                        b0 = 1 + (r0 + dh) * WP + dw - 1
                        b1 = 1 + (r0 + 4 + dh) * WP + dw - 1
                        st, sp = (tap == 0), (tap == 8)
                        w_lo = wsb[0:64, tap * 64:(tap + 1) * 64]
                        w_hi = wsb[64:128, tap * 64:(tap + 1) * 64]
                        nc.tensor.matmul(psA[0:64, :], w_lo,
                                         xpad[0:64, b0:b0 + 456],
                                         start=st, stop=sp,
                                         tile_position=(0, 0))
                        nc.tensor.matmul(psB[0:64, :], w_hi,
                                         xpad[64:128, b0:b0 + 456],
                                         start=st, stop=sp,
                                         tile_position=(64, 0))
                        nc.tensor.matmul(psA[64:128, :], w_lo,
                                         xpad[0:64, b1:b1 + 456],
                                         start=st, stop=sp,
                                         tile_position=(0, 64))
                        nc.tensor.matmul(psB[64:128, :], w_hi,
                                         xpad[64:128, b1:b1 + 456],
                                         start=st, stop=sp,
                                         tile_position=(64, 64))
                    for s, ps in ((0, psA), (1, psB)):
                        # compact 114->112 in the evac so the DMA source is
                        # contiguous: one 1792B descriptor per partition
                        osb = opool.tile([128, 448], F32, tag=f"osb{s}")
                        nc.vector.tensor_scalar(
                            osb[:].rearrange("p (r w) -> p r w", w=W),
                            ps[:].rearrange("p (r w) -> p r w", w=WP)
                            [:, :, 1:113],
                            bias_sb[:, 0:1], None,
                            op0=mybir.AluOpType.add)
                        nc.scalar.dma_start(
                            out[2 * p + s, 2 * t:2 * t + 2]
                            .rearrange("u c r w -> (u c) (r w)"),
                            osb[:])
    nc.compile()
    return nc


def _shard_inputs(x, weight):
    """Build per-core in_maps for program A."""
    xf = np.ascontiguousarray(x, dtype=np.float32).reshape(-1)
    xf = np.concatenate([xf, np.zeros(QWIN, np.float32)])
    in_maps = []
    pres = []
    for k in range(N_CORES):
        core_pre = []
        xin = np.empty((4, 128, QCOLS), np.float32)
        for j in range(4):
            s = 4 * k + j
            start = s * SAMPLE
            gstart = (start // GS) * GS
            core_pre.append(start - gstart)
            xin[j] = xf[gstart:gstart + QWIN].reshape(128, QCOLS)
        in_maps.append({"xin": xin, "w": np.ascontiguousarray(weight, np.float32)})
        pres.append(core_pre)
    return in_maps, pres


def kernel(x, weight, bias):
    from concourse.bass_utils import run_bass_kernel_spmd

    if "quant" not in _cache:
        _cache["quant"] = build_quant()
    if "conv" not in _cache:
        _cache["conv"] = build_conv()

    core_ids = list(range(N_CORES))
    trace = _trace_enabled()
    if trace:
        _install_trace_shim()

    in_maps, pres = _shard_inputs(x, weight)
    resA = run_bass_kernel_spmd(_cache["quant"], in_maps, core_ids, trace=trace)
    last_exec_ns["quant"] = resA.exec_time_ns
    last_results["quant"] = resA

    bias2 = np.concatenate([np.asarray(bias, np.float32)] * 2)
    in_maps_b = []
    for k in range(N_CORES):
        qx = np.asarray(resA.results[k]["qx"])          # [4,128,QCOLS] bf16
        qw = np.asarray(resA.results[k]["qw"]).reshape(64, 64, 9)  # [o,i,t]
        qx4 = np.zeros((4, C, WP, WP), ml_dtypes.bfloat16)
        for j in range(4):
            pre = pres[k][j]
            qx4[j, :, 1:113, 1:113] = (
                qx[j].reshape(-1)[pre:pre + SAMPLE].reshape(C, H, W))
        wtio = qw.transpose(1, 2, 0)                    # [i,t,o]
        wblk = np.concatenate([wtio, wtio], axis=0)     # [128,9,64] both halves
        in_maps_b.append({"qx4": qx4, "wblk": wblk.reshape(128, 9 * 64),
                          "bias2": bias2})
    resB = run_bass_kernel_spmd(_cache["conv"], in_maps_b, core_ids, trace=trace)
    last_exec_ns["conv"] = resB.exec_time_ns
    last_results["conv"] = resB

    out = np.concatenate(
        [np.asarray(resB.results[k]["out"]) for k in range(N_CORES)], axis=0)
    # [32, 28, C, 4, W] chunk-major -> NCHW
    out = out.transpose(0, 2, 1, 3, 4).reshape(B, C, H, W)
    return np.ascontiguousarray(out, dtype=np.float32)



# revision 14
# speedup vs baseline: 1.3516x; 1.0583x over previous
"""BFP-quantized 3x3 conv (nn_BFConv2d) on 8 TRN2 NeuronCores.

Strategy (data-parallel over batch, 4 samples/core):
  Program A (quantize): per core, for each of its 4 samples, load a
    group-aligned window of the flattened x (the BFP group grid is global
    over the flat tensor; each per-sample window starts on a 36-element
    group boundary, so the in-kernel grid is exact), compute the BFP
    quantization with the magic-number trick
        q = (x + M) - M,  M = 1.5 * 2^23 * scale = exp_bits(absmax) * 98304
    (exact round-half-even onto the group lattice; results are <=9
    significant bits so bf16 is exact), and write q as bf16. The weight
    tensor (36864 elems = exactly 1024 groups) is quantized the same way.
  Host: slice each sample's quantized window by its group-grid phase
    (pre in [0,36)) to get slab-aligned q; pure numpy, no device work.
  Program B (conv): 3x3 conv as 9 shifted 64x64 bf16 matmuls per output
    tile, using TensorE 64x64 array tiling: quadrant (0,0) processes the
    even sample of a pair (SBUF partitions 0-63), quadrant (64,64) the odd
    sample (partitions 64-127), both accumulating into one PSUM bank.
    ScalarE evacuates PSUM with the bias add fused; one full-width DMA
    writes both samples' rows (64*12544 == 802816 makes the pair layout
    contiguous in NCHW).
"""

import os
import sys
from contextlib import ExitStack

import numpy as np

sys.path.insert(0, "/opt/trn_rl_repo")

import ml_dtypes  # noqa: E402
import concourse.bacc as bacc  # noqa: E402
import concourse.mybir as mybir  # noqa: E402
import concourse.tile as tile  # noqa: E402

F32 = mybir.dt.float32
BF16 = mybir.dt.bfloat16
I32 = mybir.dt.int32

N_CORES = 8
B = 32                      # batch
C = 64                      # channels (in == out)
H = W = 112
SAMPLE = C * H * W          # 802816 elems per sample
GS = 36                     # BFP group size
GPP = 175                   # groups per partition in the quantize window
QCOLS = GPP * GS            # 6300
QWIN = 128 * QCOLS          # 806400 elems: covers a sample + phase slack
WP = W + 2                  # padded row width 114
XPAD = WP * WP + 2          # padded sample + 2 guard slots
MAGIC_MUL = 98304.0         # 1.5 * 2^16:  exp2(e) * this == 1.5*2^23*2^(e-7)

_cache = {}
last_exec_ns = {}
last_results = {}


def _ensure_snap_op():
    """Register a custom DVE op BFP_SNAP_ANT: out = (in0 + in1) - in1.

    One streaming pass for the BFP magic-number snap (vs add + subtract as
    two scalar_tensor_tensor passes). The per-NEFF DVE table machinery picks
    it up from dve_ops.OPS; sha is pinned from this environment's lowering.
    """
    import concourse.dve_ops as dops
    if getattr(dops, "_BFP_SNAP_ANT", None) is not None:
        return dops._BFP_SNAP_ANT
    from concourse.dve_spec import Spec, Src0, Src1, lower as spec_lower
    from concourse.dve_uop import DveOpSpec

    def _snap_ref(in0, in1, s0, s1, imm2):
        a = in0.astype(np.float32)
        b = np.broadcast_to(in1.astype(np.float32), in1.shape).reshape(a.shape)
        return (a + b) - b

    spec = Spec(body=(Src0 + Src1) - Src1, reference=_snap_ref)
    op = dops.DveOp("BFP_SNAP_ANT", spec, subdim=False, uops_sha={})
    idx = max(dops._SUB_OPCODE_FOR_NAME.values()) + 1
    assert idx < 0x20
    dops.OPS.append(op)
    dops.CUSTOM_DVE_SPECS["BFP_SNAP_ANT"] = spec
    dops._SUB_OPCODE_FOR_NAME["BFP_SNAP_ANT"] = idx
    for ver in ("v3", "v4"):
        try:
            s = DveOpSpec(name=op.name, opcode=idx,
                          uops=spec_lower(spec, ver=ver), rd1_en=True)
            op.uops_sha[ver] = s.sha(ver)
        except Exception:
            pass
    dops._BFP_SNAP_ANT = op
    return op


def _trace_enabled():
    return os.environ.get("BFP_TRACE") == "1"


def _install_trace_shim():
    """Provide antenv.axon_hooks (NTFF profiling hook) if the image lacks it.

    Mirrors trn_agent_boot.trn_boot._ntff_profile_via_ctypes: drives NRT
    profiling through the axon PJRT .so so run_bass_kernel_spmd(trace=True)
    can report HW exec time.
    """
    import types
    import ctypes
    import contextlib
    try:
        from antenv.axon_hooks import get_axon_ntff_profile_hook  # noqa: F401
        return
    except ImportError:
        pass
    so_path = "/opt/axon/libaxon_pjrt.so"
    if not os.path.exists(so_path):
        return
    lib = ctypes.CDLL(so_path)
    if not hasattr(lib, "axon_start_nrt_profile"):
        return
    lib.axon_start_nrt_profile.argtypes = [ctypes.POINTER(ctypes.c_int64),
                                           ctypes.c_size_t]
    lib.axon_start_nrt_profile.restype = ctypes.c_int64
    lib.axon_stop_nrt_profile.argtypes = [ctypes.c_char_p]
    lib.axon_stop_nrt_profile.restype = ctypes.c_int64

    @contextlib.contextmanager
    def _hook(output_dir, device_ids):
        import jax
        jax.devices()
        if device_ids:
            ids = (ctypes.c_int64 * len(device_ids))(*device_ids)
            rc = lib.axon_start_nrt_profile(ids, len(device_ids))
        else:
            rc = lib.axon_start_nrt_profile(None, 0)
        if rc != 0:
            raise RuntimeError(f"axon_start_nrt_profile rc={rc}")
        try:
            yield
        finally:
            n = lib.axon_stop_nrt_profile(str(output_dir).encode())
            print(f"profile: {n} ntff file(s) -> {output_dir}", file=sys.stderr)

    mod = types.ModuleType("antenv.axon_hooks")
    state = {"hook": _hook}
    mod.get_axon_ntff_profile_hook = lambda: state["hook"]
    mod.set_axon_ntff_profile_hook = lambda h: state.update(hook=h)
    sys.modules["antenv.axon_hooks"] = mod
    import antenv
    antenv.axon_hooks = mod
    from concourse import bass_utils as bu
    bu.upload_artifacts = lambda d: str(d)  # no egress from this container


I16 = mybir.dt.int16
# free-dim split of a sample window into 2 group-aligned chunks
CHUNK_COLS = (3168, 3132)          # 88 + 87 groups = 175
CHUNK_OFF = (0, 3168)


def build_quant():
    """v2: scalar casts f32->bf16; vector reduces bf16 at 2x and snaps;
    gpsimd computes per-group magic constants. Half-sample chunks."""
    snap = _ensure_snap_op()
    nc = bacc.Bacc(None)
    xin = nc.declare_dram_parameter("xin", [4, 128, QCOLS], F32, isOutput=False)
    win = nc.declare_dram_parameter("w", [C, C, 3, 3], F32, isOutput=False)
    qx = nc.declare_dram_parameter("qx", [4, 128, QCOLS], BF16, isOutput=True)
    qw = nc.declare_dram_parameter("qw", [128, 288], BF16, isOutput=True)

    def bfp_w(pool, spool, src_ap, ngroups, out_tile):
        """f32 path for the small weight tensor (exact, as v1)."""
        g3 = lambda ap: ap.rearrange("p (g s) -> p g s", s=GS)
        m = spool.tile([128, ngroups], F32, tag="wm")
        nc.vector.tensor_reduce(m[:], g3(src_ap), axis=mybir.AxisListType.X,
                                op=mybir.AluOpType.max, apply_absolute_value=True)
        mi = spool.tile([128, ngroups], I32, tag="wmi")
        nc.vector.tensor_scalar(mi[:], m[:].bitcast(I32), 0x7F800000, None,
                                op0=mybir.AluOpType.bitwise_and)
        mf = spool.tile([128, ngroups], F32, tag="wmf")
        nc.vector.tensor_scalar(mf[:], mi[:].bitcast(F32), MAGIC_MUL, None,
                                op0=mybir.AluOpType.mult)
        mb = mf[:].unsqueeze(-1).broadcast_to([128, ngroups, GS])
        nc.vector._custom_dve(snap, out=g3(out_tile[:]), in0=g3(src_ap), in1=mb)

    with tile.TileContext(nc) as tc:
        with ExitStack() as ctx:
            pool = ctx.enter_context(tc.tile_pool(name="big", bufs=3))
            spool = ctx.enter_context(tc.tile_pool(name="small", bufs=3))
            # weight first: its tiny DMA lands long before sample 0's slab
            wf = pool.tile([128, 288], F32, tag="wf")
            nc.sync.dma_start(wf[:], win[:].rearrange("o i h w -> (o i h w)")
                              .rearrange("(p c) -> p c", p=128))
            qwt = pool.tile([128, 288], BF16, tag="qwt")
            bfp_w(pool, spool, wf[:], 8, qwt)
            nc.scalar.dma_start(qw[:], qwt[:])
            xr = xin[:].rearrange("j p c -> p j c")
            qr = qx[:].rearrange("j p c -> p j c")
            for j in range(4):
                for h in range(2):
                    cols, off = CHUNK_COLS[h], CHUNK_OFF[h]
                    ng = cols // GS
                    g3 = lambda ap: ap.rearrange("p (g s) -> p g s", s=GS)
                    xs = pool.tile([128, cols], F32, tag=f"xs{h}")
                    nc.sync.dma_start(xs[:], xr[:, j, off:off + cols])
                    # scalar engine: cast to bf16 (feeds both reduce and snap)
                    xb = pool.tile([128, cols], BF16, tag=f"xb{h}")
                    nc.scalar.copy(xb[:], xs[:])
                    # vector: group abs-max on bf16 (2x mode)
                    m = spool.tile([128, ng], BF16, tag=f"m{h}")
                    nc.vector.tensor_reduce(
                        m[:], g3(xb[:]), axis=mybir.AxisListType.X,
                        op=mybir.AluOpType.max, apply_absolute_value=True)
                    # M = exp_bits(max) * 1.5*2^16  (tiny)
                    mi = spool.tile([128, ng], I16, tag=f"mi{h}")
                    nc.vector.tensor_scalar(
                        mi[:], m[:].bitcast(I16), 0x7F80, None,
                        op0=mybir.AluOpType.bitwise_and)
                    mf = spool.tile([128, ng], BF16, tag=f"mf{h}")
                    nc.vector.tensor_scalar(
                        mf[:], mi[:].bitcast(BF16), MAGIC_MUL, None,
                        op0=mybir.AluOpType.mult)
                    # vector: snap (x + M) - M in one custom-DVE pass
                    q = pool.tile([128, cols], BF16, tag=f"q{h}", bufs=6)
                    mb = mf[:].unsqueeze(-1).broadcast_to([128, ng, GS])
                    nc.vector._custom_dve(snap, out=g3(q[:]), in0=g3(xb[:]),
                                          in1=mb)
                    nc.scalar.dma_start(qr[:, j, off:off + cols], q[:])
    nc.compile()
    return nc


def build_conv():
    """v2: 4x 64x64 PE-array tiles. T0/T2 convolve sample A (row-chunks 2t,
    2t+1), T8/T10 sample B, all four concurrently; weights replicated on
    both SBUF partition halves. Two PSUM banks per round (A and B)."""
    nc = bacc.Bacc(None)
    qx4 = nc.declare_dram_parameter("qx4", [4, C, WP, WP], BF16, isOutput=False)
    wblk = nc.declare_dram_parameter("wblk", [128, 9 * 64], BF16, isOutput=False)
    bias2 = nc.declare_dram_parameter("bias2", [128], F32, isOutput=False)
    # chunk-major: [sample, 4-row chunk, c, r, w]; host transposes to NCHW.
    # Keeps each out-DMA one call with 128 contiguous 1792B descriptors.
    out = nc.declare_dram_parameter("out", [4, 28, C, 4, W], F32, isOutput=True)

    with tile.TileContext(nc) as tc:
        with ExitStack() as ctx:
            consts = ctx.enter_context(tc.tile_pool(name="consts", bufs=1))
            xpool = ctx.enter_context(tc.tile_pool(name="x", bufs=2))
            opool = ctx.enter_context(tc.tile_pool(name="o", bufs=4))
            psum = ctx.enter_context(tc.tile_pool(name="ps", bufs=2, space="PSUM"))

            # wblk[p, tap*64 + o]: W[tap][ic=p%64, oc=o], same on both halves
            wsb = consts.tile([128, 9 * 64], BF16)
            nc.sync.dma_start(wsb[:], wblk[:])
            bias_sb = consts.tile([128, 1], F32)
            nc.sync.dma_start(bias_sb[:], bias2[:, None])

            for p in range(2):
                xpad = xpool.tile([128, XPAD], BF16, tag="xpad")
                nc.gpsimd.memset(xpad[:, 0:1], 0.0)           # guard slots
                nc.gpsimd.memset(xpad[:, XPAD - 1:XPAD], 0.0)
                # load in 4 row-bands so round 0 can start after band 0;
                # band k covers padded rows [29k, 29k+29) (+1 trailing row)
                for k in range(4):
                    rlo = 29 * k
                    nrows = 29 if k < 3 else 27
                    nc.sync.dma_start(
                        xpad[0:64, 1 + rlo * WP:1 + (rlo + nrows) * WP],
                        qx4[2 * p, :, rlo:rlo + nrows, :]
                        .rearrange("c h w -> c (h w)"))
                    nc.sync.dma_start(
                        xpad[64:128, 1 + rlo * WP:1 + (rlo + nrows) * WP],
                        qx4[2 * p + 1, :, rlo:rlo + nrows, :]
                        .rearrange("c h w -> c (h w)"))

                for t in range(14):
                    r0 = 8 * t
                    psA = psum.tile([128, 456], F32, tag="psA")
                    psB = psum.tile([128, 456], F32, tag="psB")
                    for tap in range(9):
                        dh, dw = divmod(tap, 3)
                        b0 = 1 + (r0 + dh) * WP + dw - 1
                        b1 = 1 + (r0 + 4 + dh) * WP + dw - 1
                        st, sp = (tap == 0), (tap == 8)
                        w_lo = wsb[0:64, tap * 64:(tap + 1) * 64]
                        w_hi = wsb[64:128, tap * 64:(tap + 1) * 64]
                        nc.tensor.matmul(psA[0:64, :], w_lo,
                                         xpad[0:64, b0:b0 + 456],
                                         start=st, stop=sp,
                                         tile_position=(0, 0))
                        nc.tensor.matmul(psB[0:64, :], w_hi,
                                         xpad[64:128, b0:b0 + 456],
                                         start=st, stop=sp,
                                         tile_position=(64, 0))
                        nc.tensor.matmul(psA[64:128, :], w_lo,
                                         xpad[0:64, b1:b1 + 456],
                                         start=st, stop=sp,
                                         tile_position=(0, 64))
                        nc.tensor.matmul(psB[64:128, :], w_hi,
                                         xpad[64:128, b1:b1 + 456],
                                         start=st, stop=sp,
                                         tile_position=(64, 64))
                    for s, ps in ((0, psA), (1, psB)):
                        # compact 114->112 in the evac so the DMA source is
                        # contiguous: one 1792B descriptor per partition
                        osb = opool.tile([128, 448], F32, tag=f"osb{s}")
                        nc.vector.tensor_scalar(
                            osb[:].rearrange("p (r w) -> p r w", w=W),
                            ps[:].rearrange("p (r w) -> p r w", w=WP)
                            [:, :, 1:113],
                            bias_sb[:, 0:1], None,
                            op0=mybir.AluOpType.add)
                        nc.scalar.dma_start(
                            out[2 * p + s, 2 * t:2 * t + 2]
                            .rearrange("u c r w -> (u c) (r w)"),
                            osb[:])
    nc.compile()
    return nc


def _shard_inputs(x, weight):
    """Build per-core in_maps for program A."""
    xf = np.ascontiguousarray(x, dtype=np.float32).reshape(-1)
    xf = np.concatenate([xf, np.zeros(QWIN, np.float32)])
    in_maps = []
    pres = []
    for k in range(N_CORES):
        core_pre = []
        xin = np.empty((4, 128, QCOLS), np.float32)
        for j in range(4):
            s = 4 * k + j
            start = s * SAMPLE
            gstart = (start // GS) * GS
            core_pre.append(start - gstart)
            xin[j] = xf[gstart:gstart + QWIN].reshape(128, QCOLS)
        in_maps.append({"xin": xin, "w": np.ascontiguousarray(weight, np.float32)})
        pres.append(core_pre)
    return in_maps, pres


def kernel(x, weight, bias):
    from concourse.bass_utils import run_bass_kernel_spmd

    if "quant" not in _cache:
        _cache["quant"] = build_quant()
    if "conv" not in _cache:
        _cache["conv"] = build_conv()

    core_ids = list(range(N_CORES))
    trace = _trace_enabled()
    if trace:
        _install_trace_shim()

    in_maps, pres = _shard_inputs(x, weight)
    resA = run_bass_kernel_spmd(_cache["quant"], in_maps, core_ids, trace=trace)
    last_exec_ns["quant"] = resA.exec_time_ns
    last_results["quant"] = resA

    bias2 = np.concatenate([np.asarray(bias, np.float32)] * 2)
    in_maps_b = []
    for k in range(N_CORES):
        qx = np.asarray(resA.results[k]["qx"])          # [4,128,QCOLS] bf16
        qw = np.asarray(resA.results[k]["qw"]).reshape(64, 64, 9)  # [o,i,t]
        qx4 = np.zeros((4, C, WP, WP), ml_dtypes.bfloat16)
        for j in range(4):
            pre = pres[k][j]
            qx4[j, :, 1:113, 1:113] = (
                qx[j].reshape(-1)[pre:pre + SAMPLE].reshape(C, H, W))
        wtio = qw.transpose(1, 2, 0)                    # [i,t,o]
        wblk = np.concatenate([wtio, wtio], axis=0)     # [128,9,64] both halves
        in_maps_b.append({"qx4": qx4, "wblk": wblk.reshape(128, 9 * 64),
                          "bias2": bias2})
    resB = run_bass_kernel_spmd(_cache["conv"], in_maps_b, core_ids, trace=trace)
    last_exec_ns["conv"] = resB.exec_time_ns
    last_results["conv"] = resB

    out = np.concatenate(
        [np.asarray(resB.results[k]["out"]) for k in range(N_CORES)], axis=0)
    # [32, 28, C, 4, W] chunk-major -> NCHW
    out = out.transpose(0, 2, 1, 3, 4).reshape(B, C, H, W)
    return np.ascontiguousarray(out, dtype=np.float32)

